# revision 1
# baseline (speedup 1.0000x reference)
"""BitLinear (BitNet a4.x-style) Trainium2 kernel.

Computes  out = act_quant_int4(x) @ ste_ternary(w).T  for
x:[8192,4096] f32, w:[4096,4096] f32, on 8 NeuronCores.

Math structure exploited:
  - act_quant_int4(x) rows are  k/s_t  with integer k in [-7,7],
    s_t = 7/amax_t  (per-token).  The clip to [-8,7] is a no-op since
    |x*s| <= 7 by construction.
  - ste_ternary(w) = q * scale with q in {-1,0,1},
    scale = max(mean|w|, 1e-8)  (global scalar).
  - So out[t,o] = (scale * amax_t / 7) * sum_i k[t,i] * q[o,i].
    The inner sum is an exact small-integer dot product computed on the
    PE array with fp8 DoubleRow matmuls (exact fp32 PSUM accumulation);
    rows are scaled by f_t = scale*amax_t/7 during PSUM eviction and
    written out in bf16 (host widens to f32; ~2e-3 rel err, well under
    the 2e-2 gate).

Three launches on 8 cores (an exact global ternary scale is required --
approximate per-shard scales measurably fail the accuracy gate, and
on-device collectives are far too expensive under this fabric):
  1. wscale: per-core partial |w| sums over a 512-row shard of wT in
     128-element chunks; host finishes the reduction in f64.
  2. wquant: ternarize the shard with the exact scale (clamp chain on
     DVE/Pool, magic-constant rounding, fp8 cast on ACT), writing wq8
     directly in the pair-interleaved DRAM layout main consumes.
  3. main, data-parallel over tokens: per 128-token tile: amax
     (DVE+Pool) -> s=7/amax, f=amax*scale/7 (DVE) -> y=x*s+MAGIC (Pool)
     -> fp8 k via ACT bias=-MAGIC -> activation transpose (bf16-bitcast
     PE transposes, bit-preserving, plus DMA-xbar for a couple of tiles
     to balance engines) -> ACT pair-shuffle eviction to kt[c,b,t] ->
     plain-DoubleRow fp8 matmuls (probe-verified [p,b,m] mapping, no
     SwInterleave reversal) accumulating 16 contraction planes into
     rotating PSUM bank pairs per 1024-feature stripe -> DVE eviction
     with *f to bf16 -> store.  Work is emitted in predicted-ready
     order so the PE stays fed while x tiles and wq stripes stream.
"""

import numpy as np
from contextlib import ExitStack

import concourse.bacc as bacc
import concourse.bass as bass
import concourse.mybir as mybir
import concourse.tile as tile
from concourse.bass_utils import run_bass_kernel_spmd

F32 = mybir.dt.float32
FP8 = mybir.dt.float8e4
BF16 = mybir.dt.bfloat16
ALU = mybir.AluOpType
ACTF = mybir.ActivationFunctionType
DR = mybir.MatmulPerfMode.DoubleRow

TOK, DIN, DOUT = 8192, 4096, 4096
NCORES = 8
TSH = TOK // NCORES      # 1024 tokens per core
NT = TSH // 128          # 8 token tiles per core
WSEG = DIN // NCORES     # 512 wT rows per core in the w launches
NPLANES = DIN // 256     # 16 DoubleRow contraction planes
NOC = 8                  # 512-wide feature chunks
NSTRIPE = 4              # 1024-wide feature stripes (2 chunks each)
MAGIC = 12582912.0       # 1.5*2^23: float add/sub round-to-nearest-int
CLAMP = float(np.nextafter(np.float32(1.5), np.float32(0.0)))
EPS = 1e-8

_CACHE = {}


def _build_wscale_nc():
    """Per-core partial |w| sums over the 512-row shard, in 128-element
    chunks (fp32 accumulation error ~1e-7; host finishes in f64)."""
    nc = bacc.Bacc("TRN2", target_bir_lowering=False, debug=False,
                   num_devices=NCORES)
    wseg = nc.dram_tensor("wseg", [WSEG, DIN], F32,
                          kind="ExternalInput").ap()
    psums = nc.dram_tensor("psums", [128, 128], F32,
                           kind="ExternalOutput").ap()
    with tile.TileContext(nc) as tc, ExitStack() as ctx:
        pool = ctx.enter_context(tc.tile_pool(name="w", bufs=4))
        spool = ctx.enter_context(tc.tile_pool(name="s", bufs=1))
        sums = spool.tile([128, 8, 16], F32)
        chunks = []
        for i in range(8):
            r0, c0 = (i // 2) * 128, (i % 2) * 2048
            if i < 6:
                chunks.append((r0, c0, 2048, i * 16, 16))
            else:
                chunks.append((r0, c0, 1024, i * 16, 8))
                chunks.append((r0, c0 + 1024, 1024, i * 16 + 8, 8))
        sums_f = sums.rearrange("p a b -> p (a b)")
        for r0, c0, w, o0, nb in chunks:
            wt = pool.tile([128, nb, 128], F32,
                           tag="wt" if nb == 16 else "wth",
                           name=f"wt{o0}")
            nc.sync.dma_start(
                out=wt,
                in_=wseg[r0:r0 + 128, c0:c0 + w].rearrange(
                    "p (a b) -> p a b", a=nb))
            nc.vector.tensor_reduce(
                out=sums_f[:, o0:o0 + nb], in_=wt,
                axis=mybir.AxisListType.X,
                op=ALU.add, apply_absolute_value=True)
        nc.sync.dma_start(out=psums, in_=sums.rearrange("p a b -> p (a b)"))
    nc.compile()
    return nc


def _build_wquant_nc():
    """Ternarize the [512, 4096] shard of wT into fp8, writing straight
    into the pair-interleaved layout the main launch consumes:
      wq8[s_local, p, b, o] = q[i, o],  i = c*512 + s_local*256 + 2p + b.
    Shard row r = rb*128 + rr maps to (s_local=rb//2, p=64*(rb%2)+rr//2,
    b=rr%2).  Stores are issued from ACT right after the cast so the SP
    load stream is never blocked."""
    nc = bacc.Bacc("TRN2", target_bir_lowering=False, debug=False,
                   num_devices=NCORES)
    wseg = nc.dram_tensor("wseg", [WSEG, DIN], F32,
                          kind="ExternalInput").ap()
    sca = nc.dram_tensor("sca", [128, 2], F32, kind="ExternalInput").ap()
    wq8 = nc.dram_tensor("wq8", [2, 128, 2, DIN], FP8,
                         kind="ExternalOutput").ap()
    with tile.TileContext(nc) as tc, ExitStack() as ctx:
        const = ctx.enter_context(tc.tile_pool(name="const", bufs=1))
        pool = ctx.enter_context(tc.tile_pool(name="w", bufs=4))
        qpool = ctx.enter_context(tc.tile_pool(name="q", bufs=4))
        scat = const.tile([128, 2], F32)
        nc.sync.dma_start(out=scat, in_=sca)
        units = []
        for i in range(8):
            rb, h = i // 2, i % 2
            if i < 6:
                units.append((rb, h * 2048, 2048))
            else:
                units.append((rb, h * 2048, 1024))
                units.append((rb, h * 2048 + 1024, 1024))
        for i, (rb, c0, w) in enumerate(units):
            r0 = rb * 128
            wt = pool.tile([128, w], F32, tag="wt" if w == 2048 else "wth",
                           name=f"wt{i}")
            nc.sync.dma_start(out=wt, in_=wseg[r0:r0 + 128, c0:c0 + w])
            nc.vector.tensor_scalar(
                out=wt, in0=wt, scalar1=scat[:, 0:1], scalar2=CLAMP,
                op0=ALU.mult, op1=ALU.min)
            eng2 = nc.gpsimd if i % 2 == 0 else nc.vector
            eng2.tensor_scalar(
                out=wt, in0=wt, scalar1=-CLAMP, scalar2=MAGIC,
                op0=ALU.max, op1=ALU.add)
            qt = qpool.tile([128, w], FP8, tag="qt" if w == 2048 else "qth",
                            name=f"qt{i}")
            nc.scalar.activation(out=qt, in_=wt, func=ACTF.Copy,
                                 bias=-MAGIC, scale=1.0)
            nc.scalar.dma_start(
                out=wq8[rb // 2, 64 * (rb % 2):64 * (rb % 2) + 64, :,
                        c0:c0 + w].rearrange("p b o -> (p b) o"),
                in_=qt)
    nc.compile()
    return nc


def _build_main_nc(timing=None, record=None, load_order=None,
                   shuf_eng="act"):
    """Main data-parallel launch; see module docstring section 3."""
    nc = bacc.Bacc("TRN2", target_bir_lowering=False, debug=False,
                   num_devices=NCORES)
    xs = nc.dram_tensor("xs", [TSH, DIN], F32, kind="ExternalInput").ap()
    # Pair-interleaved quantized weights: wts8[s, p, b, o] = q[s*256+2p+b, o]
    wts8 = nc.dram_tensor("wts8", [NPLANES, 128, 2, DOUT], FP8,
                          kind="ExternalInput").ap()
    sca = nc.dram_tensor("sca", [128, 1], F32, kind="ExternalInput").ap()
    idt = nc.dram_tensor("idt", [128, 128], BF16, kind="ExternalInput").ap()
    out = nc.dram_tensor("out", [TSH, DOUT], BF16, kind="ExternalOutput").ap()

    with tile.TileContext(nc) as tc, ExitStack() as ctx:
        const = ctx.enter_context(tc.tile_pool(name="const", bufs=1))
        wqpool = ctx.enter_context(tc.tile_pool(name="wqp", bufs=NOC))
        xpool = ctx.enter_context(tc.tile_pool(name="xp", bufs=2))
        k8pool = ctx.enter_context(tc.tile_pool(name="k8p", bufs=2))
        ktpool = ctx.enter_context(tc.tile_pool(name="ktp", bufs=2 * NT))
        smalls = ctx.enter_context(tc.tile_pool(name="smalls", bufs=2 * NT))
        opool = ctx.enter_context(tc.tile_pool(name="osb", bufs=3))
        accpool = ctx.enter_context(
            tc.tile_pool(name="acc", bufs=3, space="PSUM"))
        tpspool = ctx.enter_context(
            tc.tile_pool(name="tps", bufs=2, space="PSUM"))

        scat = const.tile([128, 1], F32)
        nc.sync.dma_start(out=scat, in_=sca)
        ident = const.tile([128, 128], BF16)
        nc.sync.dma_start(out=ident, in_=idt)

        # ---- emission event list, ordered by predicted ready time ----
        events = []   # (time, seq, kind, payload)
        seq = [0]

        def emit(t, kind, payload):
            if timing is not None:
                t = timing.get((kind, payload), t)
            events.append((t, seq[0], kind, payload))
            seq[0] += 1

        XL = 5.825   # x tile load [128, 4096] f32
        OCL = 5.825  # wq oc-chunk load (two b-half DMAs)
        if load_order is None:
            load_order = [("x", 0), ("oc", 0), ("oc", 1)]
            nx = 1
            for c in range(2, NOC):
                load_order.append(("x", nx)); nx += 1
                load_order.append(("oc", c))
            while nx < NT:
                load_order.append(("x", nx)); nx += 1
        tdma = 0.0
        t_x = [0.0] * NT
        t_oc = [0.0] * NOC
        for kind, i in load_order:
            if kind == "x":
                tdma += XL
                emit(tdma - XL, "xload", i)
                t_x[i] = tdma
            else:
                tdma += OCL
                emit(tdma - OCL, "ocload", i)
                t_oc[i] = tdma
        t_kt = [0.0] * NT
        for t in range(NT):
            # chain: amax (DVE, 4.3+sem) -> schain (3 small DVE) ->
            # quant halves (Pool) -> cast halves (ACT) -> PE transposes
            # -> shuffles (ACT)
            emit(t_x[t] + 0.02, "amax", t)
            emit(t_x[t] + 5.6, "schain", t)
            emit(t_x[t] + 6.8, "fpool", t)
            emit(t_x[t] + 6.6, "quant_h", (t, 0))
            emit(t_x[t] + 7.0, "quant_h", (t, 1))
            emit(t_x[t] + 8.5, "cast_h", (t, 0))
            emit(t_x[t] + 10.4, "cast_h", (t, 1))
            emit(t_x[t] + 9.5, "tr", (t, 0))
            emit(t_x[t] + 11.6, "tr", (t, 1))
            emit(t_x[t] + 10.9, "shuf", (t, 0))
            emit(t_x[t] + 12.8, "shuf", (t, 1))
            t_kt[t] = t_x[t] + 13.0
        # matmul passes: stripe p of tile t
        passes = []
        for t in range(NT):
            for p in range(NSTRIPE):
                rdy = max(t_kt[t], t_oc[2 * p + 1] + 1.0)
                passes.append((rdy, t, p))
        passes.sort()
        pe_t = 0.0
        for rdy, t, p in passes:
            pe_t = max(pe_t, rdy) + 3.6
            emit(pe_t - 3.6 + 1e-3, "pass", (t, p))
            emit(pe_t + 0.9, "evict", (t, p))

        # Topological fix-up: whatever the (possibly measured) times say,
        # an event may not be emitted before events that create the tile
        # objects it references.
        tmap = {}
        for tt, sq, kind, payload in events:
            tmap[(kind, payload)] = tt

        def bump(key, *prereqs):
            lo = max((tmap[k] for k in prereqs if k in tmap), default=None)
            if lo is not None and tmap[key] <= lo:
                tmap[key] = lo + 1e-4
        for t in range(NT):
            bump(("amax", t), ("xload", t))
            bump(("schain", t), ("amax", t))
            bump(("fpool", t), ("amax", t))
            for h in range(2):
                bump(("quant_h", (t, h)), ("schain", t))
                bump(("cast_h", (t, h)), ("quant_h", (t, h)))
                bump(("tr", (t, h)), ("cast_h", (t, h)))
                bump(("shuf", (t, h)), ("tr", (t, h)))
        for t in range(NT):
            for p in range(NSTRIPE):
                bump(("pass", (t, p)), ("schain", t),
                     ("ocload", 2 * p), ("ocload", 2 * p + 1))
                bump(("evict", (t, p)), ("pass", (t, p)), ("fpool", t))
        events = [(tmap[(kind, payload)], sq, kind, payload)
                  for tt, sq, kind, payload in events]

        # ---- state built during emission ----
        wq = [None] * NOC
        xt = [None] * NT
        k8 = [None] * NT
        kt = [[None, None] for _ in range(NT)]
        f_ap = [None] * NT
        s_ap = [None] * NT
        sm_t = [None] * NT
        acc_tiles = {}
        trbuf = {}

        for _, _, kind, payload in sorted(events):
            _n0 = nc.next_id() if record is not None else 0
            if kind == "xload":
                t = payload
                xt[t] = xpool.tile([128, DIN], F32, tag="xt", name=f"xt{t}")
                nc.sync.dma_start(
                    out=xt[t], in_=xs[t * 128:(t + 1) * 128, :])
            elif kind == "ocload":
                c = payload
                wq[c] = wqpool.tile([128, NPLANES, 2, 512], FP8, tag="wq",
                                    name=f"wq{c}")
                for b in range(2):
                    nc.sync.dma_start(
                        out=wq[c][:, :, b, :],
                        in_=wts8.rearrange("s p b o -> p s b o")[
                            :, :, b, c * 512:(c + 1) * 512])
            elif kind == "amax":
                t = payload
                sm = smalls.tile([128, 4], F32, tag="sch", name=f"sch{t}")
                sm_t[t] = sm
                s_ap[t] = sm[:, 2:3]
                f_ap[t] = sm[:, 3:4]
                nc.vector.tensor_reduce(
                    out=sm[:, 0:1], in_=xt[t],
                    axis=mybir.AxisListType.X, op=ALU.max,
                    apply_absolute_value=True)
                for h2 in range(2):
                    kt[t][h2] = ktpool.tile([128, 8, 2, 128], FP8,
                                            tag="kt", name=f"kt{t}_{h2}")
            elif kind == "schain":
                t = payload
                sm = sm_t[t]
                amax = sm[:, 0:1]
                nc.vector.tensor_scalar_mul(sm[:, 1:2], amax, 1.0 / 7.0)
                nc.vector.reciprocal(out=sm[:, 2:3], in_=sm[:, 1:2])
                s_ap[t] = sm[:, 2:3]
            elif kind == "fpool":
                t = payload
                sm = sm_t[t]
                nc.gpsimd.tensor_scalar(
                    out=f_ap[t], in0=sm[:, 0:1], scalar1=scat[:, 0:1],
                    scalar2=None, op0=ALU.mult)
            elif kind == "quant_h":
                t, h = payload
                eng = nc.gpsimd
                eng.tensor_scalar(
                    out=xt[t][:, h * 2048:(h + 1) * 2048],
                    in0=xt[t][:, h * 2048:(h + 1) * 2048],
                    scalar1=s_ap[t], scalar2=MAGIC,
                    op0=ALU.mult, op1=ALU.add)
            elif kind == "cast_h":
                t, h = payload
                if h == 0:
                    k8[t] = k8pool.tile([128, DIN], FP8, tag="k8",
                                        name=f"k8_{t}")
                eng = nc.gpsimd if h == 0 else nc.vector
                eng.tensor_scalar(
                    out=k8[t][:, h * 2048:(h + 1) * 2048],
                    in0=xt[t][:, h * 2048:(h + 1) * 2048],
                    scalar1=-MAGIC, scalar2=None, op0=ALU.add)
            elif kind == "tr":
                t, h = payload
                k16 = k8[t].bitcast(BF16)  # [128, 2048] u16-pairs
                tps = tpspool.tile([128, 8, 128], BF16, tag="tps",
                                   name=f"tps{t}_{h}")
                trbuf[(t, h)] = tps
                for gi in range(8):
                    nc.tensor.transpose(
                        tps[:, gi, :],
                        k16[:, h * 1024 + gi * 128:
                            h * 1024 + (gi + 1) * 128],
                        ident)

            elif kind == "shuf":
                t, h = payload
                if shuf_eng == "act" or (shuf_eng == "split" and h == 0):
                    nc.scalar.activation(
                        out=kt[t][h],
                        in_=trbuf[(t, h)].bitcast(FP8).rearrange(
                            "p g (t b) -> p g b t", b=2),
                        func=ACTF.Copy, bias=0.0, scale=1.0)
                else:
                    nc.vector.tensor_copy(
                        out=kt[t][h],
                        in_=trbuf[(t, h)].bitcast(FP8).rearrange(
                            "p g (t b) -> p g b t", b=2))
            elif kind == "pass":
                t, p = payload
                acc = accpool.tile([128, 1024], F32, tag="acc",
                                   name=f"acc{t}_{p}")
                acc_tiles[(t, p)] = acc
                for st in range(NPLANES):
                    lhsT = kt[t][st // 8][:, st % 8, :, :]
                    for j in range(2):
                        nc.tensor.matmul(
                            acc[:, j * 512:(j + 1) * 512], lhsT,
                            wq[2 * p + j][:, st, :, :],
                            start=(st == 0), stop=(st == NPLANES - 1),
                            perf_mode=DR)
            elif kind == "evict":
                t, p = payload
                acc = acc_tiles[(t, p)]
                ot = opool.tile([128, 1024], BF16, tag="osb",
                                name=f"osb{t}_{p}")
                nc.scalar.activation(
                    out=ot, in_=acc, func=ACTF.Copy, bias=0.0,
                    scale=f_ap[t])
                nc.scalar.dma_start(
                    out=out[t * 128:(t + 1) * 128,
                            p * 1024:(p + 1) * 1024],
                    in_=ot)
            if record is not None:
                record.append((kind, payload,
                               [f"I-{i}" for i in range(_n0 + 1,
                                                        nc.next_id())]))

    nc.compile()
    return nc


def _get_ncs():
    if "wscale" not in _CACHE:
        _CACHE["wscale"] = _build_wscale_nc()
    if "wquant" not in _CACHE:
        _CACHE["wquant"] = _build_wquant_nc()
    if "main" not in _CACHE:
        _CACHE["main"] = _build_main_nc()
    return _CACHE["wscale"], _CACHE["wquant"], _CACHE["main"]


def kernel(x: np.ndarray, latent_weight: np.ndarray,
           _collect=None) -> np.ndarray:
    x = np.ascontiguousarray(x, dtype=np.float32)
    wT = np.ascontiguousarray(latent_weight.T.astype(np.float32))
    nc_scale, nc_wq, nc_main = _get_ncs()
    core_ids = list(range(NCORES))
    fp8np = mybir.dt.np(FP8)
    bf16np = mybir.dt.np(BF16)

    segs = [wT[c * WSEG:(c + 1) * WSEG, :] for c in core_ids]
    in1 = [{"wseg": segs[c]} for c in core_ids]
    r1 = run_bass_kernel_spmd(nc_scale, in1, core_ids=core_ids)
    total = np.float64(0.0)
    for c in core_ids:
        total += r1.results[c]["psums"].astype(np.float64).sum()
    mean = np.float32(total / (DIN * DOUT))
    scale = np.maximum(mean, np.float32(EPS))
    inv_scale = np.float32(1.0) / scale

    sca2 = np.empty((128, 2), dtype=np.float32)
    sca2[:, 0] = inv_scale
    sca2[:, 1] = scale
    in2 = [{"wseg": segs[c], "sca": sca2} for c in core_ids]
    r2 = run_bass_kernel_spmd(nc_wq, in2, core_ids=core_ids)
    # wts8[s, p, b, o] with s = 2c + s_local: concat per-core outputs
    wts8 = np.ascontiguousarray(
        np.concatenate([r2.results[c]["wq8"] for c in core_ids], axis=0))

    sca = np.full((128, 1), scale / np.float32(7.0), dtype=np.float32)
    idt = np.eye(128, dtype=np.float32).astype(bf16np)
    in3 = [{"xs": x[c * TSH:(c + 1) * TSH, :], "wts8": wts8, "sca": sca,
            "idt": idt} for c in core_ids]
    r3 = run_bass_kernel_spmd(nc_main, in3, core_ids=core_ids)

    outp = np.empty((TOK, DOUT), dtype=np.float32)
    for c in core_ids:
        outp[c * TSH:(c + 1) * TSH, :] = \
            r3.results[c]["out"].astype(np.float32)
    if _collect is not None:
        _collect["r1"] = r1
        _collect["r2"] = r2
        _collect["r3"] = r3
    return outp



# revision 10
# speedup vs baseline: 1.1622x; 1.1622x over previous
"""BitLinear (BitNet a4.x-style) Trainium2 kernel.

Computes  out = act_quant_int4(x) @ ste_ternary(w).T  for
x:[8192,4096] f32, w:[4096,4096] f32, on 8 NeuronCores.

Math structure exploited:
  - act_quant_int4(x) rows are  k/s_t  with integer k in [-7,7],
    s_t = 7/amax_t  (per-token).  The clip to [-8,7] is a no-op since
    |x*s| <= 7 by construction.
  - ste_ternary(w) = q * scale with q in {-1,0,1},
    scale = max(mean|w|, 1e-8)  (global scalar, computed exactly on
    host in f64 -- it is a single scalar reduction).
  - So out[t,o] = (scale * amax_t / 7) * sum_i k[t,i] * q[o,i].
    The inner sum is an exact small-integer dot product computed on the
    PE array with fp8 DoubleRow matmuls (exact fp32 PSUM accumulation);
    rows are scaled by f_t = scale*amax_t/7 during PSUM eviction and
    written out in bf16 (host widens to f32; ~2e-3 rel err, well under
    the 2e-2 gate).

Two launches on 8 cores:
  1. wquant: ternarize a [512, 4096] shard of wT into fp8 with the
     exact host scale, writing the pair-interleaved DRAM layout main
     consumes.  16 [128,1024] units pipelined over DVE/Pool/ACT.
  2. main, data-parallel over tokens.  Per core: stream 8 x tiles
     (f32) and 8 wq feature-chunks (fp8) through a serialized DMA
     device; per 128-token tile run a quarter-grained quant pipeline
     (amax split DVE/Pool -> s, f -> y=x*s+MAGIC -> fp8 via -MAGIC ->
     PE transpose (bf16 bitcast) -> ACT pair shuffle) and issue
     availability-ordered [128t x 512o] matmul passes (16 DoubleRow
     fp8 matmuls each) with DVE/ACT evictions scaled by f_t.
     Emission order is self-tuned: build with predicted times, measure
     with the timeline cost model, re-emit in measured order.
"""

import numpy as np
from contextlib import ExitStack

import concourse.bacc as bacc
import concourse.bass as bass
import concourse.mybir as mybir
import concourse.tile as tile
from concourse.bass_utils import run_bass_kernel_spmd

F32 = mybir.dt.float32
FP8 = mybir.dt.float8e4
BF16 = mybir.dt.bfloat16
ALU = mybir.AluOpType
ACTF = mybir.ActivationFunctionType
DR = mybir.MatmulPerfMode.DoubleRow

TOK, DIN, DOUT = 8192, 4096, 4096
NCORES = 8
TSH = TOK // NCORES      # 1024 tokens per core
NT = TSH // 128          # 8 token tiles per core
WSEG = DIN // NCORES     # 512 wT rows per core in the wquant launch
NPLANES = DIN // 256     # 16 DoubleRow contraction planes
NOC = 8                  # 512-wide feature chunks
MAGIC = 12582912.0       # 1.5*2^23: float add/sub round-to-nearest-int
CLAMP = float(np.nextafter(np.float32(1.5), np.float32(0.0)))
EPS = 1e-8

_CACHE = {}


def _build_wquant_nc(timing=None, record=None):
    """Ternarize the [512, 4096] shard of wT into fp8, writing straight
    into the pair-interleaved layout the main launch consumes:
      wq8[s_local, p, b, o] = q[i, o],  i = c*512 + s_local*256 + 2p + b.
    Shard row r = rb*128 + rr maps to (s_local=rb//2, p=64*(rb%2)+rr//2,
    b=rr%2).  16 [128,1024] units: load -> mult+min (DVE/Pool alt) ->
    max+add MAGIC (other engine) -> fp8 cast on ACT -> store."""
    nc = bacc.Bacc("TRN2", target_bir_lowering=False, debug=False,
                   num_devices=NCORES)
    wseg = nc.dram_tensor("wseg", [WSEG, DIN], F32,
                          kind="ExternalInput").ap()
    sca = nc.dram_tensor("sca", [128, 2], F32, kind="ExternalInput").ap()
    wq8 = nc.dram_tensor("wq8", [2, 128, 2, DIN], FP8,
                         kind="ExternalOutput").ap()
    NU = 16  # units: (rb in 0..3) x (cq in 0..3), each [128, 1024]
    with tile.TileContext(nc) as tc, ExitStack() as ctx:
        const = ctx.enter_context(tc.tile_pool(name="const", bufs=1))
        pool = ctx.enter_context(tc.tile_pool(name="w", bufs=6))
        qpool = ctx.enter_context(tc.tile_pool(name="q", bufs=16))

        events = []
        seq = [0]

        def emit(t, kind, payload):
            if timing is not None:
                t = timing.get((kind, payload), t)
            events.append((t, seq[0], kind, payload))
            seq[0] += 1

        UL = 1.456  # [128, 1024] f32 load
        emit(-1.0, "consts", 0)
        for u in range(NU):
            emit(u * UL, "load", u)
            emit(u * UL + UL + 0.1, "ts1", u)
            emit(u * UL + UL + 0.85, "ts2", u)
            emit(u * UL + UL + 1.7, "cast", u)
            emit(max(u * UL + UL + 2.9, 16 * UL + 0.1 + 0.37 * u),
                 "store", u)

        tmap = {}
        for tt, sq, kind, payload in events:
            tmap[(kind, payload)] = tt

        def bump(key, *prereqs):
            lo = max((tmap[k] for k in prereqs if k in tmap), default=None)
            if lo is not None and tmap[key] <= lo:
                tmap[key] = lo + 1e-4
        for u in range(NU):
            bump(("ts1", u), ("load", u), ("consts", 0))
            bump(("ts2", u), ("ts1", u))
            bump(("cast", u), ("ts2", u))
            bump(("store", u), ("cast", u))
        events = [(tmap[(kind, payload)], sq, kind, payload)
                  for tt, sq, kind, payload in events]

        state = {}
        for _, _, kind, payload in sorted(events):
            _n0 = nc.next_id() if record is not None else 0
            if kind == "consts":
                scat = const.tile([128, 2], F32)
                state["scat"] = scat
                nc.sync.dma_start(out=scat, in_=sca)
            elif kind == "load":
                u = payload
                rb, cq = u // 4, u % 4
                wt = pool.tile([128, 1024], F32, tag="wt", name=f"wt{u}")
                state[u] = wt
                nc.sync.dma_start(
                    out=wt,
                    in_=wseg[rb * 128:(rb + 1) * 128,
                             cq * 1024:(cq + 1) * 1024])
            elif kind == "ts1":
                u = payload
                eng = nc.vector
                eng.tensor_scalar(
                    out=state[u], in0=state[u],
                    scalar1=state["scat"][:, 0:1], scalar2=CLAMP,
                    op0=ALU.mult, op1=ALU.min)
            elif kind == "ts2":
                u = payload
                eng = nc.gpsimd if (u % 2 == 0 and u < 12) else nc.vector
                eng.tensor_scalar(
                    out=state[u], in0=state[u],
                    scalar1=-CLAMP, scalar2=MAGIC,
                    op0=ALU.max, op1=ALU.add)
            elif kind == "cast":
                u = payload
                qt = qpool.tile([128, 1024], FP8, tag="qt", name=f"qt{u}")
                state[("q", u)] = qt
                nc.scalar.activation(out=qt, in_=state[u], func=ACTF.Copy,
                                     bias=-MAGIC, scale=1.0)
            elif kind == "store":
                u = payload
                rb, cq = u // 4, u % 4
                nc.gpsimd.dma_start(
                    out=wq8[rb // 2, 64 * (rb % 2):64 * (rb % 2) + 64, :,
                            cq * 1024:(cq + 1) * 1024].rearrange(
                                "p b o -> (p b) o"),
                    in_=state[("q", u)])
            if record is not None:
                record.append((kind, payload,
                               [f"I-{i}" for i in range(_n0 + 1,
                                                        nc.next_id())]))
    nc.compile()
    return nc


def _build_main_nc(timing=None, record=None):
    """Main data-parallel launch; see module docstring."""
    nc = bacc.Bacc("TRN2", target_bir_lowering=False, debug=False,
                   num_devices=NCORES)
    xs = nc.dram_tensor("xs", [TSH, DIN], F32, kind="ExternalInput").ap()
    # Pair-interleaved quantized weights: wts8[s, p, b, o] = q[s*256+2p+b, o]
    wts8 = nc.dram_tensor("wts8", [NPLANES, 128, 2, DOUT], FP8,
                          kind="ExternalInput").ap()
    sca = nc.dram_tensor("sca", [128, 1], F32, kind="ExternalInput").ap()
    idt = nc.dram_tensor("idt", [128, 128], BF16, kind="ExternalInput").ap()
    out = nc.dram_tensor("out", [TSH, DOUT], BF16, kind="ExternalOutput").ap()

    wts8_p = wts8.rearrange("s p b o -> p s b o")

    with tile.TileContext(nc) as tc, ExitStack() as ctx:
        const = ctx.enter_context(tc.tile_pool(name="const", bufs=1))
        wqpool = ctx.enter_context(tc.tile_pool(name="wqp", bufs=NOC))
        xpool = ctx.enter_context(tc.tile_pool(name="xp", bufs=2))
        k8pool = ctx.enter_context(tc.tile_pool(name="k8p", bufs=2))
        ktpool = ctx.enter_context(tc.tile_pool(name="ktp", bufs=2 * NT))
        smalls = ctx.enter_context(tc.tile_pool(name="smalls", bufs=2 * NT))
        opool = ctx.enter_context(tc.tile_pool(name="osb", bufs=7))
        accpool = ctx.enter_context(
            tc.tile_pool(name="acc", bufs=6, space="PSUM"))
        tpspool = ctx.enter_context(
            tc.tile_pool(name="tps", bufs=2, space="PSUM"))

        # ---- emission event list, ordered by predicted ready time ----
        events = []
        seq = [0]

        def emit(t, kind, payload):
            if timing is not None:
                t = timing.get((kind, payload), t)
            events.append((t, seq[0], kind, payload))
            seq[0] += 1

        XQ = 5.825 / 4   # x tile quarter load [128, 1024] f32
        OCL = 5.825      # wq oc-chunk load
        # Alternating load order x0 c0 x1 c1 ... ; x tiles stream as 4
        # quarter-DMAs each so amax can start incrementally.
        tdma = 0.0
        t_x = [0.0] * NT          # full-tile land time
        t_xq = [[0.0] * 4 for _ in range(NT)]
        t_oc = [0.0] * NOC
        for q in range(4):
            emit(tdma - 0.001, "xload", (0, q))
            tdma += XQ
            t_xq[0][q] = tdma
        t_x[0] = tdma
        emit(0.01, "consts", 0)
        order = []
        for i in range(1, NT):
            order.append(("oc", i - 1))
            order.append(("x", i))
        order.append(("oc", NOC - 1))
        for kind, i in order:
            if kind == "x":
                for q in range(4):
                    emit(tdma - 0.001, "xload", (i, q))
                    tdma += XQ
                    t_xq[i][q] = tdma
                t_x[i] = tdma
            else:
                emit(tdma - 0.001, "ocload", i)
                tdma += OCL
                t_oc[i] = tdma

        # per-tile quant chain: amax per loaded quarter (DVE), combine,
        # then quant+cast quarters on DVE (q0,q1) / Pool (q2,q3)
        t_ktq = [[0.0] * 4 for _ in range(NT)]   # kt quarter ready
        for t in range(NT):
            for q in range(4):
                emit(t_xq[t][q] + 0.02, "amaxq", (t, q))
            T0 = t_x[t]
            emit(T0 + 1.35, "comb", t)           # DVE: s = 1/(amax/7)
            emit(T0 + 1.65, "fq", t)             # Pool: f = amax*sc/7
            emit(T0 + 1.70, "quant", (t, 0))
            emit(T0 + 2.80, "cast", (t, 0))
            emit(T0 + 3.90, "quant", (t, 1))
            emit(T0 + 5.00, "cast", (t, 1))
            emit(T0 + 1.75, "quant", (t, 2))
            emit(T0 + 2.90, "cast", (t, 2))
            emit(T0 + 4.05, "quant", (t, 3))
            emit(T0 + 5.20, "cast", (t, 3))
            cready = [T0 + 3.9, T0 + 6.1, T0 + 4.1, T0 + 6.4]
            for q in range(4):
                emit(cready[q] + 0.15, "tr", (t, q))
                emit(cready[q] + 0.55, "shuf", (t, q))
                t_ktq[t][q] = cready[q] + 1.75

        # matmul passes in availability order
        passes = []
        for t in range(NT):
            for c in range(NOC):
                rdy = max(max(t_ktq[t]), t_oc[c] + 0.9)
                passes.append((rdy, t, c))
        passes.sort()
        pe_t = 0.0
        for rdy, t, c in passes:
            pe_t = max(pe_t, rdy) + 1.71
            emit(pe_t - 1.71 + 1e-3, "pass", (t, c))
            emit(pe_t + 0.25, "evict", (t, c))
            emit(pe_t + 1.05, "store", (t, c))

        # Topological fix-up: an event may not be emitted before events
        # that create the tile objects it references.
        tmap = {}
        for tt, sq, kind, payload in events:
            tmap[(kind, payload)] = tt

        def bump(key, *prereqs):
            lo = max((tmap[k] for k in prereqs if k in tmap), default=None)
            if lo is not None and tmap[key] <= lo:
                tmap[key] = lo + 1e-4
        for t in range(NT):
            for q in range(4):
                bump(("amaxq", (t, q)), ("xload", (t, q)))
            bump(("comb", t), ("amaxq", (t, 0)), ("amaxq", (t, 1)),
                 ("amaxq", (t, 2)), ("amaxq", (t, 3)))
            bump(("fq", t), ("comb", t), ("consts", 0))
            for q in range(4):
                bump(("quant", (t, q)), ("comb", t), ("xload", (t, q)))
                bump(("cast", (t, q)), ("quant", (t, q)))
                bump(("tr", (t, q)), ("cast", (t, q)), ("consts", 0))
                bump(("shuf", (t, q)), ("tr", (t, q)))
        for t in range(NT):
            for c in range(NOC):
                bump(("pass", (t, c)),
                     ("shuf", (t, 0)), ("shuf", (t, 1)),
                     ("shuf", (t, 2)), ("shuf", (t, 3)),
                     ("ocload", c))
                bump(("evict", (t, c)), ("pass", (t, c)), ("fq", t))
                bump(("store", (t, c)), ("evict", (t, c)))
        events = [(tmap[(kind, payload)], sq, kind, payload)
                  for tt, sq, kind, payload in events]

        # ---- state built during emission ----
        wq = [None] * NOC
        xt = [None] * NT
        k8 = [None] * NT
        kt = [[None, None] for _ in range(NT)]
        f_ap = [None] * NT
        s_ap = [None] * NT
        sm_t = [None] * NT
        acc_tiles = {}
        trbuf = {}
        ot_tiles = {}
        scat = [None]
        ident = [None]

        for _, _, kind, payload in sorted(events):
            _n0 = nc.next_id() if record is not None else 0
            if kind == "consts":
                scat[0] = const.tile([128, 1], F32, name="scat")
                nc.sync.dma_start(out=scat[0], in_=sca)
                ident[0] = const.tile([128, 128], BF16, name="ident")
                nc.sync.dma_start(out=ident[0], in_=idt)
            elif kind == "xload":
                t, q = payload
                if q == 0:
                    xt[t] = xpool.tile([128, DIN], F32, tag="xt",
                                       name=f"xt{t}")
                nc.sync.dma_start(
                    out=xt[t][:, q * 1024:(q + 1) * 1024],
                    in_=xs[t * 128:(t + 1) * 128,
                           q * 1024:(q + 1) * 1024])
            elif kind == "ocload":
                c = payload
                wq[c] = wqpool.tile([128, NPLANES, 2, 512], FP8, tag="wq",
                                    name=f"wq{c}")
                for b in range(2):
                    nc.sync.dma_start(
                        out=wq[c][:, :, b, :],
                        in_=wts8_p[:, :, b, c * 512:(c + 1) * 512])
            elif kind == "amaxq":
                t, q = payload
                if q == 0:
                    sm = smalls.tile([128, 8], F32, tag="sch",
                                     name=f"sch{t}")
                    sm_t[t] = sm
                    s_ap[t] = sm[:, 5:6]
                    f_ap[t] = sm[:, 6:7]
                    for h2 in range(2):
                        kt[t][h2] = ktpool.tile([128, 8, 2, 128], FP8,
                                                tag="kt",
                                                name=f"kt{t}_{h2}")
                    k8[t] = k8pool.tile([128, DIN], FP8, tag="k8",
                                        name=f"k8_{t}")
                nc.vector.tensor_reduce(
                    out=sm_t[t][:, q:q + 1],
                    in_=xt[t][:, q * 1024:(q + 1) * 1024],
                    axis=mybir.AxisListType.X, op=ALU.max,
                    apply_absolute_value=True)
            elif kind == "comb":
                t = payload
                sm = sm_t[t]
                nc.vector.tensor_reduce(
                    out=sm[:, 4:5], in_=sm[:, 0:4],
                    axis=mybir.AxisListType.X, op=ALU.max)
                nc.vector.tensor_scalar_mul(sm[:, 7:8], sm[:, 4:5],
                                            1.0 / 7.0)
                nc.vector.reciprocal(out=sm[:, 5:6], in_=sm[:, 7:8])
            elif kind == "fq":
                t = payload
                nc.gpsimd.tensor_scalar(
                    out=f_ap[t], in0=sm_t[t][:, 4:5],
                    scalar1=scat[0][:, 0:1], scalar2=None, op0=ALU.mult)
            elif kind == "quant":
                t, q = payload
                eng = nc.vector if q < 2 else nc.gpsimd
                eng.tensor_scalar(
                    out=xt[t][:, q * 1024:(q + 1) * 1024],
                    in0=xt[t][:, q * 1024:(q + 1) * 1024],
                    scalar1=s_ap[t], scalar2=MAGIC,
                    op0=ALU.mult, op1=ALU.add)
            elif kind == "cast":
                t, q = payload
                eng = nc.vector if q < 2 else nc.gpsimd
                eng.tensor_scalar(
                    out=k8[t][:, q * 1024:(q + 1) * 1024],
                    in0=xt[t][:, q * 1024:(q + 1) * 1024],
                    scalar1=-MAGIC, scalar2=None, op0=ALU.add)
            elif kind == "tr":
                t, q = payload
                h = q // 2
                k16 = k8[t].bitcast(BF16)  # [128, 2048] u16-pairs
                if q % 2 == 0:
                    tps = tpspool.tile([128, 8, 128], BF16, tag="tps",
                                       name=f"tps{t}_{h}")
                    trbuf[(t, h)] = tps
                tps = trbuf[(t, h)]
                for gi in range(4):
                    g = (q % 2) * 4 + gi   # group within half
                    nc.tensor.transpose(
                        tps[:, g, :],
                        k16[:, h * 1024 + g * 128:
                            h * 1024 + (g + 1) * 128],
                        ident[0])
            elif kind == "shuf":
                t, q = payload
                h = q // 2
                g0 = (q % 2) * 4
                nc.scalar.activation(
                    out=kt[t][h][:, g0:g0 + 4, :, :],
                    in_=trbuf[(t, h)][:, g0:g0 + 4, :].bitcast(
                        FP8).rearrange("p g (t b) -> p g b t", b=2),
                    func=ACTF.Copy, bias=0.0, scale=1.0)
            elif kind == "pass":
                t, c = payload
                acc = accpool.tile([128, 512], F32, tag="acc",
                                   name=f"acc{t}_{c}")
                acc_tiles[(t, c)] = acc
                for st in range(NPLANES):
                    lhsT = kt[t][st // 8][:, st % 8, :, :]
                    nc.tensor.matmul(
                        acc, lhsT, wq[c][:, st, :, :],
                        start=(st == 0), stop=(st == NPLANES - 1),
                        perf_mode=DR)
            elif kind == "evict":
                t, c = payload
                acc = acc_tiles[(t, c)]
                ot = opool.tile([128, 512], BF16, tag="osb",
                                name=f"osb{t}_{c}")
                ot_tiles[(t, c)] = ot
                nc.scalar.activation(
                    out=ot, in_=acc, func=ACTF.Copy, bias=0.0,
                    scale=f_ap[t])
            elif kind == "store":
                t, c = payload
                eng = nc.scalar if (t * NOC + c) % 2 == 0 else nc.sync
                eng.dma_start(
                    out=out[t * 128:(t + 1) * 128,
                            c * 512:(c + 1) * 512],
                    in_=ot_tiles[(t, c)])
            if record is not None:
                record.append((kind, payload,
                               [f"I-{i}" for i in range(_n0 + 1,
                                                        nc.next_id())]))

    nc.compile()
    return nc


def _get_ncs():
    if "wquant" not in _CACHE:
        _CACHE["wquant"] = _build_wquant_nc()
    if "main" not in _CACHE:
        _CACHE["main"] = _build_main_nc()
    return _CACHE["wquant"], _CACHE["main"]


def kernel(x: np.ndarray, latent_weight: np.ndarray,
           _collect=None) -> np.ndarray:
    x = np.ascontiguousarray(x, dtype=np.float32)
    wT = np.ascontiguousarray(latent_weight.T.astype(np.float32))
    nc_wq, nc_main = _get_ncs()
    core_ids = list(range(NCORES))
    bf16np = mybir.dt.np(BF16)

    # Exact global ternary scale (host; a single scalar reduction in f64,
    # matching the baseline's host-side f64 finish of device partials).
    mean = np.float32(
        np.abs(latent_weight).astype(np.float64).sum() / (DIN * DOUT))
    scale = np.maximum(mean, np.float32(EPS))
    inv_scale = np.float32(1.0) / scale

    segs = [wT[c * WSEG:(c + 1) * WSEG, :] for c in core_ids]
    sca2 = np.empty((128, 2), dtype=np.float32)
    sca2[:, 0] = inv_scale
    sca2[:, 1] = scale
    in2 = [{"wseg": segs[c], "sca": sca2} for c in core_ids]
    r2 = run_bass_kernel_spmd(nc_wq, in2, core_ids=core_ids)
    # wts8[s, p, b, o] with s = 2c + s_local: concat per-core outputs
    wts8 = np.ascontiguousarray(
        np.concatenate([r2.results[c]["wq8"] for c in core_ids], axis=0))

    sca = np.full((128, 1), scale / np.float32(7.0), dtype=np.float32)
    idt = np.eye(128, dtype=np.float32).astype(bf16np)
    in3 = [{"xs": x[c * TSH:(c + 1) * TSH, :], "wts8": wts8, "sca": sca,
            "idt": idt} for c in core_ids]
    r3 = run_bass_kernel_spmd(nc_main, in3, core_ids=core_ids)

    outp = np.empty((TOK, DOUT), dtype=np.float32)
    for c in core_ids:
        outp[c * TSH:(c + 1) * TSH, :] = \
            r3.results[c]["out"].astype(np.float32)
    if _collect is not None:
        _collect["r2"] = r2
        _collect["r3"] = r3
    return outp


# revision 43
# speedup vs baseline: 1.2664x; 1.0896x over previous
"""BitLinear (BitNet a4.x-style) Trainium2 kernel.

Computes  out = act_quant_int4(x) @ ste_ternary(w).T  for
x:[8192,4096] f32, w:[4096,4096] f32, on 8 NeuronCores.

Math structure exploited:
  - act_quant_int4(x) rows are  k/s_t  with integer k in [-7,7],
    s_t = 7/amax_t  (per-token).  The clip to [-8,7] is a no-op since
    |x*s| <= 7 by construction.
  - ste_ternary(w) = q * scale with q in {-1,0,1},
    scale = max(mean|w|, 1e-8)  (global scalar, computed exactly on
    host in f64 -- it is a single scalar reduction).
  - So out[t,o] = (scale * amax_t / 7) * sum_i k[t,i] * q[o,i].
    The inner sum is an exact small-integer dot product computed on the
    PE array with fp8 DoubleRow matmuls (exact fp32 PSUM accumulation);
    rows are scaled by f_t = scale*amax_t/7 during PSUM eviction and
    written out in bf16 (host widens to f32; ~2e-3 rel err, well under
    the 2e-2 gate).

Two launches on 8 cores:
  1. wquant: ternarize a [512, 4096] shard of wT into fp8 with the
     exact host scale, writing the pair-interleaved DRAM layout main
     consumes.  32 [128,512] units pipelined over DVE (mult+min,
     max+add) and ACT (fp8 cast into per-rowblock staging tiles);
     4 coalesced stores keep the DMA device solid end to end.
  2. main, data-parallel over tokens.  Per core: stream 8 x tiles
     (f32) and 8 wq feature-chunks (fp8) through a serialized DMA
     device; per 128-token tile run a quarter-grained quant pipeline
     (amax split DVE/Pool -> s, f -> y=x*s+MAGIC -> fp8 via -MAGIC ->
     PE transpose (bf16 bitcast) -> ACT pair shuffle) and issue
     availability-ordered [128t x 512o] matmul passes (16 DoubleRow
     fp8 matmuls each) with ACT evictions scaled by f_t.
     Emission order is self-tuned: build with predicted times, measure
     with the timeline cost model, re-emit in measured order.
"""

import numpy as np
from contextlib import ExitStack

import concourse.bacc as bacc
import concourse.bass as bass
import concourse.mybir as mybir
import concourse.tile as tile
from concourse.bass_utils import run_bass_kernel_spmd

F32 = mybir.dt.float32
FP8 = mybir.dt.float8e4
BF16 = mybir.dt.bfloat16
ALU = mybir.AluOpType
ACTF = mybir.ActivationFunctionType
DR = mybir.MatmulPerfMode.DoubleRow

TOK, DIN, DOUT = 8192, 4096, 4096
NCORES = 8
TSH = TOK // NCORES      # 1024 tokens per core
NT = TSH // 128          # 8 token tiles per core
WSEG = DIN // NCORES     # 512 wT rows per core in the wquant launch
NPLANES = DIN // 256     # 16 DoubleRow contraction planes
NOC = 8                  # 512-wide feature chunks
MAGIC = 12582912.0       # 1.5*2^23: float add/sub round-to-nearest-int
CLAMP = float(np.nextafter(np.float32(1.5), np.float32(0.0)))
EPS = 1e-8

_CACHE = {}


def _build_wquant_nc(timing=None, record=None):
    """Ternarize the [512, 4096] shard of wT into fp8, writing straight
    into the pair-interleaved layout the main launch consumes:
      wq8[s_local, p, b, o] = q[i, o],  i = c*512 + s_local*256 + 2p + b.
    Shard row r = rb*128 + rr maps to (s_local=rb//2, p=64*(rb%2)+rr//2,
    b=rr%2).  16 [128,1024] units: load -> mult+min (DVE) -> max+add
    MAGIC (DVE/Pool) -> fp8 cast on ACT -> store via SP so stores queue
    behind all loads in the DMA-engine FIFO (no load starvation)."""
    nc = bacc.Bacc("TRN2", target_bir_lowering=False, debug=False,
                   num_devices=NCORES)
    wseg = nc.dram_tensor("wseg", [WSEG, DIN], F32,
                          kind="ExternalInput").ap()
    sca = nc.dram_tensor("sca", [128, 2], F32, kind="ExternalInput").ap()
    wq8 = nc.dram_tensor("wq8", [2, 128, 2, DIN], FP8,
                         kind="ExternalOutput").ap()
    NU = 32  # units: (rb in 0..3) x (cq in 0..7), each [128, 512]
    with tile.TileContext(nc) as tc, ExitStack() as ctx:
        const = ctx.enter_context(tc.tile_pool(name="const", bufs=1))
        pool = ctx.enter_context(tc.tile_pool(name="w", bufs=10))
        qpool = ctx.enter_context(tc.tile_pool(name="q", bufs=4))

        events = []
        seq = [0]

        def emit(t, kind, payload):
            if timing is not None:
                t = timing.get((kind, payload), t)
            events.append((t, seq[0], kind, payload))
            seq[0] += 1

        UL = 0.728  # [128, 512] f32 load
        emit(-1.0, "consts", 0)
        for u in range(NU):
            emit(u * UL, "load", u)
            emit(u * UL + UL + 0.1, "ts1", u)
            emit(u * UL + UL + 0.55, "ts2", u)
            emit(u * UL + UL + 1.0, "cast", u)
            if u % 8 == 7:
                emit(max(u * UL + UL + 1.8, 32 * UL + 0.1 + 1.5 * (u // 8)),
                     "store", u // 8)

        tmap = {}
        for tt, sq, kind, payload in events:
            tmap[(kind, payload)] = tt

        def bump(key, *prereqs):
            lo = max((tmap[k] for k in prereqs if k in tmap), default=None)
            if lo is not None and tmap[key] <= lo:
                tmap[key] = lo + 1e-4
        for u in range(NU):
            bump(("ts1", u), ("load", u), ("consts", 0))
            bump(("ts2", u), ("ts1", u))
            bump(("cast", u), ("ts2", u))
        for rb in range(4):
            bump(("store", rb), *[("cast", 8 * rb + j) for j in range(8)])
        events = [(tmap[(kind, payload)], sq, kind, payload)
                  for tt, sq, kind, payload in events]

        state = {}
        for _, _, kind, payload in sorted(events):
            _n0 = nc.next_id() if record is not None else 0
            if kind == "consts":
                scat = const.tile([128, 2], F32)
                state["scat"] = scat
                nc.gpsimd.dma_start(out=scat, in_=sca)
            elif kind == "load":
                u = payload
                rb, cq = u // 8, u % 8
                wt = pool.tile([128, 512], F32, tag="wt", name=f"wt{u}")
                state[u] = wt
                nc.sync.dma_start(
                    out=wt,
                    in_=wseg[rb * 128:(rb + 1) * 128,
                             cq * 512:(cq + 1) * 512])
            elif kind == "ts1":
                u = payload
                eng = nc.vector
                eng.tensor_scalar(
                    out=state[u], in0=state[u],
                    scalar1=state["scat"][:, 0:1], scalar2=CLAMP,
                    op0=ALU.mult, op1=ALU.min)
            elif kind == "ts2":
                u = payload
                eng = nc.gpsimd if (u % 2 == 0 and u < 12) else nc.vector
                eng.tensor_scalar(
                    out=state[u], in0=state[u],
                    scalar1=-CLAMP, scalar2=MAGIC,
                    op0=ALU.max, op1=ALU.add)
            elif kind == "cast":
                u = payload
                rb, cq = u // 8, u % 8
                if cq == 0:
                    state[("q", rb)] = qpool.tile(
                        [128, DIN], FP8, tag="qt", name=f"qs{rb}")
                nc.scalar.activation(
                    out=state[("q", rb)][:, cq * 512:(cq + 1) * 512],
                    in_=state[u], func=ACTF.Copy, bias=-MAGIC, scale=1.0)
            elif kind == "store":
                rb = payload
                nc.sync.dma_start(
                    out=wq8[rb // 2, 64 * (rb % 2):64 * (rb % 2) + 64, :,
                            :].rearrange("p b o -> (p b) o"),
                    in_=state[("q", rb)])
            if record is not None:
                record.append((kind, payload,
                               [f"I-{i}" for i in range(_n0 + 1,
                                                        nc.next_id())]))
    nc.compile()
    return nc


def _build_main_nc(timing=None, record=None):
    """Main data-parallel launch; see module docstring."""
    nc = bacc.Bacc("TRN2", target_bir_lowering=False, debug=False,
                   num_devices=NCORES)
    xs = nc.dram_tensor("xs", [TSH, DIN], F32, kind="ExternalInput").ap()
    # Pair-interleaved quantized weights: wts8[s, p, b, o] = q[s*256+2p+b, o]
    wts8 = nc.dram_tensor("wts8", [NPLANES, 128, 2, DOUT], FP8,
                          kind="ExternalInput").ap()
    sca = nc.dram_tensor("sca", [128, 1], F32, kind="ExternalInput").ap()
    idt = nc.dram_tensor("idt", [128, 128], BF16, kind="ExternalInput").ap()
    out = nc.dram_tensor("out", [TSH, DOUT], BF16, kind="ExternalOutput").ap()

    wts8_p = wts8.rearrange("s p b o -> p s b o")

    with tile.TileContext(nc) as tc, ExitStack() as ctx:
        const = ctx.enter_context(tc.tile_pool(name="const", bufs=1))
        wqpool = ctx.enter_context(tc.tile_pool(name="wqp", bufs=NOC))
        xpool = ctx.enter_context(tc.tile_pool(name="xp", bufs=2))
        k8pool = ctx.enter_context(tc.tile_pool(name="k8p", bufs=2))
        ktpool = ctx.enter_context(tc.tile_pool(name="ktp", bufs=2 * NT))
        smalls = ctx.enter_context(tc.tile_pool(name="smalls", bufs=NT))
        opool = ctx.enter_context(tc.tile_pool(name="osb", bufs=7))
        accpool = ctx.enter_context(
            tc.tile_pool(name="acc", bufs=6, space="PSUM"))
        tpspool = ctx.enter_context(
            tc.tile_pool(name="tps", bufs=2, space="PSUM"))

        # ---- emission event list, ordered by predicted ready time ----
        events = []
        seq = [0]

        def emit(t, kind, payload):
            if timing is not None:
                t = timing.get((kind, payload), t)
            events.append((t, seq[0], kind, payload))
            seq[0] += 1

        XQ = 5.825 / 4   # x tile quarter load [128, 1024] f32
        OCL = 5.825      # wq oc-chunk load
        # Alternating load order x0 c0 x1 c1 ... ; x tiles stream as 4
        # quarter-DMAs each so amax can start incrementally.
        tdma = 0.0
        t_x = [0.0] * NT          # full-tile land time
        t_xq = [[0.0] * 4 for _ in range(NT)]
        t_oc = [0.0] * NOC
        for q in range(4):
            emit(tdma - 0.001, "xload", (0, q))
            tdma += XQ
            t_xq[0][q] = tdma
        t_x[0] = tdma
        emit(0.01, "consts", 0)
        order = [("x", 1)]
        for i in range(2, NT):
            order.append(("oc", i - 2))
            order.append(("x", i))
        order.append(("oc", NOC - 3))
        order.append(("oc", NOC - 2))
        order.append(("oc", NOC - 1))
        for kind, i in order:
            if kind == "x":
                for q in range(4):
                    emit(tdma - 0.001, "xload", (i, q))
                    tdma += XQ
                    t_xq[i][q] = tdma
                t_x[i] = tdma
            else:
                emit(tdma - 0.001, "ocload", i)
                tdma += OCL
                t_oc[i] = tdma

        # per-tile quant chain: amax per loaded quarter (DVE), combine,
        # then quant+cast quarters on DVE (q0,q1) / Pool (q2,q3)
        t_ktq = [[0.0] * 4 for _ in range(NT)]   # kt quarter ready
        for t in range(NT):
            for q in range(4):
                emit(t_xq[t][q] + 0.02, "amaxq", (t, 2 * q))
                emit(t_xq[t][q] + 0.03, "amaxq", (t, 2 * q + 1))
            T0 = t_x[t]
            emit(T0 + 1.35, "comb", t)           # DVE: s = 1/(amax/7)
            emit(T0 + 1.65, "fq", t)             # Pool: f = amax*sc/7
            emit(T0 + 1.70, "quant", (t, 0))
            emit(T0 + 2.30, "cast", (t, 0))
            emit(T0 + 2.90, "quant", (t, 1))
            emit(T0 + 3.50, "cast", (t, 1))
            emit(T0 + 4.10, "quant", (t, 2))
            emit(T0 + 4.70, "cast", (t, 2))
            emit(T0 + 1.75, "quant", (t, 3))
            emit(T0 + 3.30, "cast", (t, 3))
            cready = [T0 + 2.9, T0 + 4.1, T0 + 5.3, T0 + 4.9]
            for q in range(4):
                emit(cready[q] + 0.15, "tr", (t, q))
                emit(cready[q] + 0.55, "shuf", (t, q))
                t_ktq[t][q] = cready[q] + 1.75

        # matmul passes in availability order
        passes = []
        for t in range(NT):
            for c in range(NOC):
                rdy = max(max(t_ktq[t]), t_oc[c] + 0.9)
                passes.append((rdy, t, c))
        passes.sort()
        pe_t = 0.0
        for rdy, t, c in passes:
            pe_t = max(pe_t, rdy) + 1.71
            emit(pe_t - 1.71 + 1e-3, "pass", (t, c))
            emit(pe_t + 0.25, "evict", (t, c))
            emit(pe_t + 1.05, "store", (t, c))

        # Topological fix-up: an event may not be emitted before events
        # that create the tile objects it references.
        tmap = {}
        for tt, sq, kind, payload in events:
            tmap[(kind, payload)] = tt

        def bump(key, *prereqs):
            lo = max((tmap[k] for k in prereqs if k in tmap), default=None)
            if lo is not None and tmap[key] <= lo:
                tmap[key] = lo + 1e-4
        for t in range(NT):
            for e in range(8):
                bump(("amaxq", (t, e)), ("xload", (t, e // 2)))
            bump(("comb", t), *[("amaxq", (t, e)) for e in range(8)])
            bump(("fq", t), ("comb", t), ("consts", 0))
            for q in range(4):
                bump(("quant", (t, q)), ("comb", t), ("xload", (t, q)))
                bump(("cast", (t, q)), ("quant", (t, q)))
                bump(("tr", (t, q)), ("cast", (t, q)), ("consts", 0),
                     *([("tr", (t, q - 1))] if q % 2 == 1 else []))
                bump(("shuf", (t, q)), ("tr", (t, q)))
        for t in range(NT):
            for c in range(NOC):
                bump(("pass", (t, c)),
                     ("shuf", (t, 0)), ("shuf", (t, 1)),
                     ("shuf", (t, 2)), ("shuf", (t, 3)),
                     ("ocload", (c, 0)), ("ocload", (c, 1)))
                bump(("evict", (t, c)), ("pass", (t, c)), ("fq", t))
                bump(("store", (t, c)), ("evict", (t, c)))
        events = [(tmap[(kind, payload)], sq, kind, payload)
                  for tt, sq, kind, payload in events]

        # ---- state built during emission ----
        wq = [None] * NOC
        xt = [None] * NT
        k8 = [None] * NT
        kt = [[None, None] for _ in range(NT)]
        f_ap = [None] * NT
        s_ap = [None] * NT
        sm_t = [None] * NT
        acc_tiles = {}
        trbuf = {}
        ot_tiles = {}
        scat = [None]
        ident = [None]

        for _, _, kind, payload in sorted(events):
            _n0 = nc.next_id() if record is not None else 0
            if kind == "consts":
                scat[0] = const.tile([128, 1], F32, name="scat")
                nc.gpsimd.dma_start(out=scat[0], in_=sca)
                ident[0] = const.tile([128, 128], BF16, name="ident")
                nc.gpsimd.dma_start(out=ident[0], in_=idt)
            elif kind == "xload":
                t, q = payload
                if q == 0:
                    xt[t] = xpool.tile([128, DIN], F32, tag="xt",
                                       name=f"xt{t}")
                nc.sync.dma_start(
                    out=xt[t][:, q * 1024:(q + 1) * 1024],
                    in_=xs[t * 128:(t + 1) * 128,
                           q * 1024:(q + 1) * 1024])
            elif kind == "ocload":
                c, h = payload
                if h == 0:
                    wq[c] = wqpool.tile([128, NPLANES, 2, 512], FP8,
                                        tag="wq", name=f"wq{c}")
                s0, s1 = h * 8, (h + 1) * 8
                for b in range(2):
                    nc.sync.dma_start(
                        out=wq[c][:, s0:s1, b, :],
                        in_=wts8_p[:, s0:s1, b,
                                   c * 512:(c + 1) * 512])
            elif kind == "amaxq":
                t, e = payload
                if e == 0:
                    sm = smalls.tile([128, 12], F32, tag="sch",
                                     name=f"sch{t}")
                    sm_t[t] = sm
                    s_ap[t] = sm[:, 9:10]
                    f_ap[t] = sm[:, 10:11]
                    for h2 in range(2):
                        kt[t][h2] = ktpool.tile([128, 8, 2, 128], FP8,
                                                tag="kt",
                                                name=f"kt{t}_{h2}")
                    k8[t] = k8pool.tile([128, DIN], FP8, tag="k8",
                                        name=f"k8_{t}")
                nc.vector.tensor_reduce(
                    out=sm_t[t][:, e:e + 1],
                    in_=xt[t][:, e * 512:(e + 1) * 512],
                    axis=mybir.AxisListType.X, op=ALU.max,
                    apply_absolute_value=True)
            elif kind == "comb":
                t = payload
                sm = sm_t[t]
                nc.vector.tensor_reduce(
                    out=sm[:, 8:9], in_=sm[:, 0:8],
                    axis=mybir.AxisListType.X, op=ALU.max)
                nc.vector.tensor_scalar_mul(sm[:, 11:12], sm[:, 8:9],
                                            1.0 / 7.0)
                nc.vector.reciprocal(out=sm[:, 9:10], in_=sm[:, 11:12])
            elif kind == "fq":
                t = payload
                nc.gpsimd.tensor_scalar(
                    out=f_ap[t], in0=sm_t[t][:, 8:9],
                    scalar1=scat[0][:, 0:1], scalar2=None, op0=ALU.mult)
            elif kind == "quant":
                t, q = payload
                eng = nc.vector if q < 3 else nc.gpsimd
                eng.tensor_scalar(
                    out=xt[t][:, q * 1024:(q + 1) * 1024],
                    in0=xt[t][:, q * 1024:(q + 1) * 1024],
                    scalar1=s_ap[t], scalar2=MAGIC,
                    op0=ALU.mult, op1=ALU.add)
            elif kind == "cast":
                t, q = payload
                eng = nc.vector if q < 3 else nc.gpsimd
                eng.tensor_scalar(
                    out=k8[t][:, q * 1024:(q + 1) * 1024],
                    in0=xt[t][:, q * 1024:(q + 1) * 1024],
                    scalar1=-MAGIC, scalar2=None, op0=ALU.add)
            elif kind == "tr":
                t, q = payload
                h = q // 2
                k16 = k8[t].bitcast(BF16)  # [128, 2048] u16-pairs
                if q % 2 == 0:
                    tps = tpspool.tile([128, 8, 128], BF16, tag="tps",
                                       name=f"tps{t}_{h}")
                    trbuf[(t, h)] = tps
                tps = trbuf[(t, h)]
                for gi in range(4):
                    g = (q % 2) * 4 + gi   # group within half
                    nc.tensor.transpose(
                        tps[:, g, :],
                        k16[:, h * 1024 + g * 128:
                            h * 1024 + (g + 1) * 128],
                        ident[0])
            elif kind == "shuf":
                t, q = payload
                h = q // 2
                g0 = (q % 2) * 4
                nc.scalar.activation(
                    out=kt[t][h][:, g0:g0 + 4, :, :],
                    in_=trbuf[(t, h)][:, g0:g0 + 4, :].bitcast(
                        FP8).rearrange("p g (t b) -> p g b t", b=2),
                    func=ACTF.Copy, bias=0.0, scale=1.0)
            elif kind == "pass":
                t, c = payload
                acc = accpool.tile([128, 512], F32, tag="acc",
                                   name=f"acc{t}_{c}")
                acc_tiles[(t, c)] = acc
                for st in range(NPLANES):
                    lhsT = kt[t][st // 8][:, st % 8, :, :]
                    nc.tensor.matmul(
                        acc, lhsT, wq[c][:, st, :, :],
                        start=(st == 0), stop=(st == NPLANES - 1),
                        perf_mode=DR)
            elif kind == "evict":
                t, c = payload
                acc = acc_tiles[(t, c)]
                ot = opool.tile([128, 512], BF16, tag="osb",
                                name=f"osb{t}_{c}")
                ot_tiles[(t, c)] = ot
                nc.scalar.activation(
                    out=ot, in_=acc, func=ACTF.Copy, bias=0.0,
                    scale=f_ap[t])
            elif kind == "store":
                t, c = payload
                eng = nc.scalar if (t * NOC + c) % 2 == 0 else nc.sync
                eng.dma_start(
                    out=out[t * 128:(t + 1) * 128,
                            c * 512:(c + 1) * 512],
                    in_=ot_tiles[(t, c)])
            if record is not None:
                record.append((kind, payload,
                               [f"I-{i}" for i in range(_n0 + 1,
                                                        nc.next_id())]))

    nc.compile()
    return nc


def _get_ncs():
    if "wquant" not in _CACHE:
        _CACHE["wquant"] = _build_wquant_nc()
    if "main" not in _CACHE:
        _CACHE["main"] = _build_main_nc()
    return _CACHE["wquant"], _CACHE["main"]


def kernel(x: np.ndarray, latent_weight: np.ndarray,
           _collect=None) -> np.ndarray:
    x = np.ascontiguousarray(x, dtype=np.float32)
    wT = np.ascontiguousarray(latent_weight.T.astype(np.float32))
    nc_wq, nc_main = _get_ncs()
    core_ids = list(range(NCORES))
    bf16np = mybir.dt.np(BF16)

    # Exact global ternary scale (host; a single scalar reduction in f64,
    # matching the baseline's host-side f64 finish of device partials).
    mean = np.float32(
        np.abs(latent_weight).astype(np.float64).sum() / (DIN * DOUT))
    scale = np.maximum(mean, np.float32(EPS))
    inv_scale = np.float32(1.0) / scale

    segs = [wT[c * WSEG:(c + 1) * WSEG, :] for c in core_ids]
    sca2 = np.empty((128, 2), dtype=np.float32)
    sca2[:, 0] = inv_scale
    sca2[:, 1] = scale
    in2 = [{"wseg": segs[c], "sca": sca2} for c in core_ids]
    r2 = run_bass_kernel_spmd(nc_wq, in2, core_ids=core_ids)
    # wts8[s, p, b, o] with s = 2c + s_local: concat per-core outputs
    wts8 = np.ascontiguousarray(
        np.concatenate([r2.results[c]["wq8"] for c in core_ids], axis=0))

    sca = np.full((128, 1), scale / np.float32(7.0), dtype=np.float32)
    idt = np.eye(128, dtype=np.float32).astype(bf16np)
    in3 = [{"xs": x[c * TSH:(c + 1) * TSH, :], "wts8": wts8, "sca": sca,
            "idt": idt} for c in core_ids]
    r3 = run_bass_kernel_spmd(nc_main, in3, core_ids=core_ids)

    outp = np.empty((TOK, DOUT), dtype=np.float32)
    for c in core_ids:
        outp[c * TSH:(c + 1) * TSH, :] = \
            r3.results[c]["out"].astype(np.float32)
    if _collect is not None:
        _collect["r2"] = r2
        _collect["r3"] = r3
    return outp
